# revision 3
# baseline (speedup 1.0000x reference)
"""Multi-head attention (B=4, N=2048, C=1024, H=16, D=64) on 8 Trainium2 cores.

Sharding: core = b*2 + hg  (b in 0..3 batches, hg in 0..1 head-groups of 8 heads).

v3 — all-bf16 datapath (f32 PSUM accumulation, rel err ~4e-3):
  - qt/kt [128 = 2 heads x 64 d, 4 head-pairs, 2048] bf16; S^T per head via
    K=64 matmuls row-positioned at partition 0/64 (tile_position).
  - per-(qc, head) S psum [128 keys, 2 ktile, 512 q], double-buffered so
    S(g+1) overlaps exp(g) — exp is one Act instruction per [128, 1024].
  - PV: lhsT = v_aug [128 keys, 65] (64 ch + ones), moving = expS [128, 512],
    accumulated over 16 key tiles -> psum [65, 512]; row 64 = denominator.
  - normalize: DVE reciprocal of den row, broadcast across 64 partitions via
    DRAM stride-0 round trip (gpsimd.partition_broadcast corrupts on HW),
    DVE mul -> ctxt bf16.
  - projection work (K mt1-3, V, most of Q, output proj) is DRAINED into the
    attention stream so the Act engine starts exp after a ~10 us prefix
    (K mt0 + Q mt0/qc0) instead of a ~60 us serial QKV phase.
"""
import os
import sys

sys.path.insert(0, "/opt/trn_rl_repo")

import ml_dtypes
import numpy as np

import concourse.bass as bass  # noqa: F401
import concourse.tile as tile
from concourse import bacc, mybir
from concourse.bass_utils import run_bass_kernel_spmd

P = 128
B, N, C = 4, 2048, 1024
H = 16
D = 64
HG = 2                 # head groups (tensor-parallel dim)
NH = H // HG           # 8 heads per core
CH = NH * D            # 512 ctx channels per core
KO = C // P            # 8 contraction tiles for projections
NKT = N // P           # 16 key tiles
QC = 512               # query chunk (psum bank)
NQC = N // QC          # 4 query chunks
NG = NKT // 2          # 8 key-tile pair groups per (qc, head)
SCALE = D ** -0.5

f32 = mybir.dt.float32
bf16 = mybir.dt.bfloat16

np_bf16 = ml_dtypes.bfloat16

BCAST = os.environ.get("BCAST", "dram")  # "pool" | "dram"

_CACHE = {}


def _build(variant="full"):
    """Build + compile the per-core Bass program (same for all 8 cores)."""
    key = (variant, BCAST)
    if key in _CACHE:
        return _CACHE[key]

    nc = bacc.Bacc("TRN2", target_bir_lowering=False, debug=False)

    xt_d = nc.dram_tensor("xt", [KO, P, N], bf16, kind="ExternalInput").ap()
    wq_d = nc.dram_tensor("wq", [KO, P, CH], bf16, kind="ExternalInput").ap()
    wk_d = nc.dram_tensor("wk", [KO, P, CH], bf16, kind="ExternalInput").ap()
    wv_d = nc.dram_tensor("wv", [KO, P, CH], bf16, kind="ExternalInput").ap()
    wo_d = nc.dram_tensor("wo", [CH // P, P, C], bf16, kind="ExternalInput").ap()
    out_d = nc.dram_tensor("out", [N, C], f32, kind="ExternalOutput").ap()

    with tile.TileContext(nc) as tc:
        with tc.tile_pool(name="persist", bufs=1) as persist:
            _build_body(nc, tc, persist, variant,
                        xt_d, wq_d, wk_d, wv_d, wo_d, out_d)

    nc.compile()
    _CACHE[key] = nc
    return nc


def _build_body(nc, tc, persist, variant, xt_d, wq_d, wk_d, wv_d, wo_d, out_d):
    xt = persist.tile([P, KO, N], bf16, tag="xt")
    qt = persist.tile([P, CH // P, N], bf16, tag="qt")
    kt = persist.tile([P, CH // P, N], bf16, tag="kt")
    v = persist.tile([P, NKT, NH, D + 1], bf16, tag="v")
    ctxt = persist.tile([P, CH // P, N], bf16, tag="ctxt")
    wq = persist.tile([P, KO, CH], bf16, tag="wq")
    wk = persist.tile([P, KO, CH], bf16, tag="wk")
    wv = persist.tile([P, KO, CH], bf16, tag="wv")
    wo = persist.tile([P, CH // P, C], bf16, tag="wo")

    # DMA order mirrors first-use order: the prefix K projection reads
    # xt[ko] + wk[ko] in ko order; wq next (Q prefix), wv (V drains), wo last.
    for ko in range(KO):
        nc.sync.dma_start(xt[:, ko, :], xt_d[ko])
        nc.sync.dma_start(wk[:, ko, :], wk_d[ko])
    for ko in range(KO):
        nc.sync.dma_start(wq[:, ko, :], wq_d[ko])
    for ko in range(KO):
        nc.sync.dma_start(wv[:, ko, :], wv_d[ko])
    for ct in range(CH // P):
        nc.sync.dma_start(wo[:, ct, :], wo_d[ct])

    nc.vector.memset(v[:, :, :, D:D + 1], 1.0)  # softmax-denominator ones col

    with (
        tc.tile_pool(name="pst", bufs=2, space="PSUM") as pst,
        tc.tile_pool(name="ppv", bufs=2, space="PSUM") as ppv,
        tc.tile_pool(name="ppo", bufs=2, space="PSUM") as ppo,
        tc.tile_pool(name="pe", bufs=4) as pe_pool,
        tc.tile_pool(name="pnorm", bufs=2) as pnorm,
        tc.tile_pool(name="pout", bufs=4) as pout,
        tc.tile_pool(name="pdram", bufs=2, space="DRAM") as pdram,
    ):
        # ---- projection group emitters (each: one [P, QC] psum + copy) ----
        def emit_kproj(qc, mt):
            qsl = slice(qc * QC, (qc + 1) * QC)
            ps = ppo.tile([P, QC], f32, tag="po")
            for ko in range(KO):
                nc.tensor.matmul(
                    ps[:], wk[:, ko, mt * P:(mt + 1) * P], xt[:, ko, qsl],
                    start=(ko == 0), stop=(ko == KO - 1))
            nc.vector.tensor_copy(kt[:, mt, qsl], ps[:])

        def emit_qproj(qc, mt):
            qsl = slice(qc * QC, (qc + 1) * QC)
            ps = ppo.tile([P, QC], f32, tag="po")
            for ko in range(KO):
                nc.tensor.matmul(
                    ps[:], wq[:, ko, mt * P:(mt + 1) * P], xt[:, ko, qsl],
                    start=(ko == 0), stop=(ko == KO - 1))
            nc.vector.tensor_copy(qt[:, mt, qsl], ps[:])

        def emit_vproj(tt):
            tsl = slice(tt * P, (tt + 1) * P)
            ps = ppo.tile([P, QC], f32, tag="po")
            for ko in range(KO):
                nc.tensor.matmul(
                    ps[:], xt[:, ko, tsl], wv[:, ko, :],
                    start=(ko == 0), stop=(ko == KO - 1))
            nc.vector.tensor_copy(
                v[:, tt, :, 0:D], ps[:].rearrange("p (h d) -> p h d", d=D))

        if variant == "qkv":
            for mt in range(CH // P):
                for qc in range(NQC):
                    emit_kproj(qc, mt)
                    emit_qproj(qc, mt)
            for tt in range(NKT):
                emit_vproj(tt)
            ot = persist.tile([P, QC], f32, tag="dump")
            nc.vector.tensor_copy(
                ot[:].rearrange("p (h d) -> p h d", d=D), v[:, 0, :, 0:D])
            nc.sync.dma_start(out_d[0:P, 0:QC], ot[:])
            return

        # ---- serial prefix: just enough for the first head's S stream ----
        for qc in range(NQC):
            emit_kproj(qc, 0)
        emit_qproj(0, 0)

        # ---- deferred projection drain, paced through the first qc ----
        # item idx i covers (qc=i//64, h=(i//8)%8, g=i%8); S(h) needs
        # kt/qt mt=h//2; PV(item i, g) needs v tiles 2g, 2g+1.
        drain_at = {}
        for i in range(NG):               # V pairs feed PV of items 0..7
            drain_at[i] = [lambda tt=2 * i: emit_vproj(tt),
                           lambda tt=2 * i + 1: emit_vproj(tt)]
        for mt in range(1, CH // P):      # K/Q for heads 2mt..2mt+1 at idx 16mt
            base = 8 * mt
            for qc in range(NQC):
                drain_at.setdefault(base + qc, []).append(
                    lambda qc=qc, mt=mt: emit_kproj(qc, mt))
            drain_at.setdefault(base + 4, []).append(
                lambda mt=mt: emit_qproj(0, mt))
        # Q(qc+1) during qc, one mt per head boundary
        for qc in range(NQC - 1):
            for mt in range(CH // P):
                drain_at.setdefault(qc * 64 + mt * 8 + 6, []).append(
                    lambda qc=qc, mt=mt: emit_qproj(qc + 1, mt))

        def emit_outproj_group(qc, i):
            qt_i, nt = 4 * qc + i // 2, i % 2
            po = ppo.tile([P, QC], f32, tag="po")
            for ct in range(CH // P):
                nc.tensor.matmul(
                    po[:], ctxt[:, ct, qt_i * P:(qt_i + 1) * P],
                    wo[:, ct, nt * QC:(nt + 1) * QC],
                    start=(ct == 0), stop=(ct == CH // P - 1))
            ot = pout.tile([P, QC], f32, tag="ot")
            nc.vector.tensor_copy(ot[:], po[:])
            nc.sync.dma_start(
                out_d[qt_i * P:(qt_i + 1) * P, nt * QC:(nt + 1) * QC], ot[:])

        if variant != "attn":
            # out-projection of qc spread across qc+1's head boundaries
            for qc in range(NQC - 1):
                for i in range(8):
                    drain_at.setdefault((qc + 1) * 64 + i * 8 + 5, []).append(
                        lambda qc=qc, i=i: emit_outproj_group(qc, i))

        flat = [(qc, h, g)
                for qc in range(NQC) for h in range(NH) for g in range(NG)]
        st_q = []

        def emit_S(it):
            qc, h, g = it
            qsl = slice(qc * QC, (qc + 1) * QC)
            hp, b64 = h // 2, (h % 2) * D
            stS = pst.tile([P, 2, QC], f32, tag="st")
            for j in range(2):
                ik = 2 * g + j
                ksl = slice(ik * P, (ik + 1) * P)
                nc.tensor.matmul(
                    stS[:, j, :],
                    kt[b64:b64 + D, hp, ksl],
                    qt[b64:b64 + D, hp, qsl],
                    start=True, stop=True, tile_position=(b64, 0))
            st_q.append(stS)

        emit_S(flat[0])
        emit_S(flat[1])
        psc = None
        for idx, it in enumerate(flat):
            qc, h, g = it
            qsl = slice(qc * QC, (qc + 1) * QC)
            stS = st_q.pop(0)
            eS = pe_pool.tile([P, 2, QC], bf16, tag="eS")
            nc.scalar.activation(eS[:], stS[:],
                                 mybir.ActivationFunctionType.Exp, scale=SCALE)
            for fn in drain_at.pop(idx, ()):
                fn()
            if idx + 2 < len(flat):
                emit_S(flat[idx + 2])
            if g == 0:
                psc = ppv.tile([P, QC], f32, tag="psc")
            for j in range(2):
                ik = 2 * g + j
                nc.tensor.matmul(
                    psc[0:D + 1, :], v[:, ik, h, :], eS[:, j, :],
                    start=(ik == 0), stop=(ik == NKT - 1))
            if g == NG - 1:
                # normalize: reciprocal of den row (DVE), broadcast across 64
                # partitions (Pool or DRAM round trip), multiply into ctxt
                bc = pnorm.tile([P, QC], f32, tag="bc")
                if BCAST == "pool":
                    rb = pnorm.tile([P, QC], f32, tag="rb")
                    nc.vector.reciprocal_approx_fast(rb[D:D + 1, :],
                                                     psc[D:D + 1, :])
                    nc.gpsimd.partition_broadcast(bc[0:D, :], rb[D:D + 1, :],
                                                  channels=D)
                else:
                    den = pnorm.tile([P, QC], f32, tag="den")
                    nc.vector.tensor_copy(den[D:D + 1, :], psc[D:D + 1, :])
                    den_dr = pdram.tile([QC], f32, tag="den_dr")
                    nc.sync.dma_start(den_dr[:], den[D:D + 1, :])
                    bcr = pnorm.tile([P, QC], f32, tag="bcr")
                    nc.sync.dma_start(bcr[0:D, :],
                                      den_dr.partition_broadcast(D))
                    nc.vector.reciprocal_approx_fast(bc[0:D, :], bcr[0:D, :])
                nc.vector.tensor_mul(
                    ctxt[(h % 2) * D:(h % 2 + 1) * D, h // 2, qsl],
                    psc[0:D, :], bc[0:D, :])

        if variant != "attn":
            for i in range(8):
                emit_outproj_group(NQC - 1, i)

        if variant == "attn":
            ot = persist.tile([P, QC], f32, tag="dump")
            nc.vector.tensor_copy(ot[:], ctxt[:, 0, 0:QC])
            nc.sync.dma_start(out_d[0:P, 0:QC], ot[:])


def _prepare_in_maps(x, wq, wk, wv, wo):
    x = np.asarray(x, dtype=np.float32)
    ws = {}
    for hg in range(HG):
        sl = slice(hg * CH, (hg + 1) * CH)
        ws[hg] = {
            "wq": np.ascontiguousarray(np.asarray(wq)[sl, :].T).astype(
                np_bf16).reshape(KO, P, CH),
            "wk": np.ascontiguousarray(np.asarray(wk)[sl, :].T).astype(
                np_bf16).reshape(KO, P, CH),
            "wv": np.ascontiguousarray(np.asarray(wv)[sl, :].T).astype(
                np_bf16).reshape(KO, P, CH),
            "wo": np.ascontiguousarray(np.asarray(wo)[:, sl].T).astype(
                np_bf16).reshape(CH // P, P, C),
        }
    in_maps = []
    for core in range(8):
        b, hg = core // HG, core % HG
        xtb = np.ascontiguousarray(x[b].T).astype(np_bf16).reshape(KO, P, N)
        m = {"xt": xtb}
        m.update(ws[hg])
        in_maps.append(m)
    return in_maps


def kernel(x, wq, wk, wv, wo, bo):
    nc = _build()
    in_maps = _prepare_in_maps(x, wq, wk, wv, wo)
    res = run_bass_kernel_spmd(nc, in_maps, core_ids=list(range(8)))
    bo = np.asarray(bo, dtype=np.float32)
    out = np.empty((B, N, C), dtype=np.float32)
    for b in range(B):
        out[b] = res.results[2 * b]["out"] + res.results[2 * b + 1]["out"] + bo
    return out


# revision 4
# speedup vs baseline: 1.3054x; 1.3054x over previous
"""Multi-head attention (B=4, N=2048, C=1024, H=16, D=64) on 8 Trainium2 cores.

Sharding: core = b*2 + hg  (b in 0..3 batches, hg in 0..1 head-groups of 8 heads).

v3 — all-bf16 datapath (f32 PSUM accumulation, rel err ~4e-3):
  - qt/kt [128 = 2 heads x 64 d, 4 head-pairs, 2048] bf16; S^T per head via
    K=64 matmuls row-positioned at partition 0/64 (tile_position).
  - per-(qc, head) S psum [128 keys, 2 ktile, 512 q], double-buffered so
    S(g+1) overlaps exp(g) — exp is one Act instruction per [128, 1024].
  - PV: lhsT = v_aug [128 keys, 65] (64 ch + ones), moving = expS [128, 512],
    accumulated over 16 key tiles -> psum [65, 512]; row 64 = denominator.
  - normalize: DVE reciprocal of den row, broadcast across 64 partitions via
    DRAM stride-0 round trip (gpsimd.partition_broadcast corrupts on HW),
    DVE mul -> ctxt bf16.
  - projection work (K mt1-3, V, most of Q, output proj) is DRAINED into the
    attention stream so the Act engine starts exp after a ~10 us prefix
    (K mt0 + Q mt0/qc0) instead of a ~60 us serial QKV phase.
"""
import os
import sys

sys.path.insert(0, "/opt/trn_rl_repo")

import ml_dtypes
import numpy as np

import concourse.bass as bass  # noqa: F401
import concourse.tile as tile
from concourse import bacc, mybir
from concourse.bass_utils import run_bass_kernel_spmd

P = 128
B, N, C = 4, 2048, 1024
H = 16
D = 64
HG = 2                 # head groups (tensor-parallel dim)
NH = H // HG           # 8 heads per core
CH = NH * D            # 512 ctx channels per core
KO = C // P            # 8 contraction tiles for projections
NKT = N // P           # 16 key tiles
QC = 512               # query chunk (psum bank)
NQC = N // QC          # 4 query chunks
NG = NKT // 2          # 8 key-tile pair groups per (qc, head)
SCALE = D ** -0.5

f32 = mybir.dt.float32
bf16 = mybir.dt.bfloat16

np_bf16 = ml_dtypes.bfloat16

BCAST = os.environ.get("BCAST", "dram")  # "pool" | "dram"

_CACHE = {}


def _build(variant="full"):
    """Build + compile the per-core Bass program (same for all 8 cores)."""
    key = (variant, BCAST)
    if key in _CACHE:
        return _CACHE[key]

    nc = bacc.Bacc("TRN2", target_bir_lowering=False, debug=False)

    xt_d = nc.dram_tensor("xt", [KO, P, N], bf16, kind="ExternalInput").ap()
    wq_d = nc.dram_tensor("wq", [KO, P, CH], bf16, kind="ExternalInput").ap()
    wk_d = nc.dram_tensor("wk", [KO, P, CH], bf16, kind="ExternalInput").ap()
    wv_d = nc.dram_tensor("wv", [KO, P, CH], bf16, kind="ExternalInput").ap()
    wo_d = nc.dram_tensor("wo", [CH // P, P, C], bf16, kind="ExternalInput").ap()
    out_d = nc.dram_tensor("out", [N, C], f32, kind="ExternalOutput").ap()

    with tile.TileContext(nc) as tc:
        with tc.tile_pool(name="persist", bufs=1) as persist:
            _build_body(nc, tc, persist, variant,
                        xt_d, wq_d, wk_d, wv_d, wo_d, out_d)

    nc.compile()
    _CACHE[key] = nc
    return nc


def _build_body(nc, tc, persist, variant, xt_d, wq_d, wk_d, wv_d, wo_d, out_d):
    xt = persist.tile([P, KO, N], bf16, tag="xt")
    qt = persist.tile([P, CH // P, N], bf16, tag="qt")
    kt = persist.tile([P, CH // P, N], bf16, tag="kt")
    v = persist.tile([P, NKT, NH, D + 1], bf16, tag="v")
    ctxt = persist.tile([P, CH // P, N], bf16, tag="ctxt")
    wq = persist.tile([P, KO, CH], bf16, tag="wq")
    wk = persist.tile([P, KO, CH], bf16, tag="wk")
    wv = persist.tile([P, KO, CH], bf16, tag="wv")
    wo = persist.tile([P, CH // P, C], bf16, tag="wo")

    # DMA order mirrors first-use order: the prefix K projection reads
    # xt[ko] + wk[ko] in ko order; wq next (Q prefix), wv (V drains), wo last.
    for ko in range(KO):
        nc.sync.dma_start(xt[:, ko, :], xt_d[ko])
        nc.sync.dma_start(wk[:, ko, :], wk_d[ko])
    for ko in range(KO):
        nc.sync.dma_start(wq[:, ko, :], wq_d[ko])
    for ko in range(KO):
        nc.sync.dma_start(wv[:, ko, :], wv_d[ko])
    for ct in range(CH // P):
        nc.sync.dma_start(wo[:, ct, :], wo_d[ct])

    nc.vector.memset(v[:, :, :, D:D + 1], 1.0)  # softmax-denominator ones col

    with (
        tc.tile_pool(name="pst", bufs=2, space="PSUM") as pst,
        tc.tile_pool(name="ppv", bufs=1, space="PSUM") as ppv,
        tc.tile_pool(name="ppo", bufs=3, space="PSUM") as ppo,
        tc.tile_pool(name="pe", bufs=6) as pe_pool,
        tc.tile_pool(name="pnorm", bufs=2) as pnorm,
        tc.tile_pool(name="pout", bufs=4) as pout,
        tc.tile_pool(name="pdram", bufs=2, space="DRAM") as pdram,
    ):
        # ---- projection group emitters (each: one [P, QC] psum + copy) ----
        def emit_kproj(qc, mt):
            qsl = slice(qc * QC, (qc + 1) * QC)
            ps = ppo.tile([P, QC], f32, tag="po")
            for ko in range(KO):
                nc.tensor.matmul(
                    ps[:], wk[:, ko, mt * P:(mt + 1) * P], xt[:, ko, qsl],
                    start=(ko == 0), stop=(ko == KO - 1))
            nc.vector.tensor_copy(kt[:, mt, qsl], ps[:])

        def emit_qproj(qc, mt):
            qsl = slice(qc * QC, (qc + 1) * QC)
            ps = ppo.tile([P, QC], f32, tag="po")
            for ko in range(KO):
                nc.tensor.matmul(
                    ps[:], wq[:, ko, mt * P:(mt + 1) * P], xt[:, ko, qsl],
                    start=(ko == 0), stop=(ko == KO - 1))
            nc.vector.tensor_copy(qt[:, mt, qsl], ps[:])

        def emit_vproj(tt):
            tsl = slice(tt * P, (tt + 1) * P)
            ps = ppo.tile([P, QC], f32, tag="po")
            for ko in range(KO):
                nc.tensor.matmul(
                    ps[:], xt[:, ko, tsl], wv[:, ko, :],
                    start=(ko == 0), stop=(ko == KO - 1))
            nc.vector.tensor_copy(
                v[:, tt, :, 0:D], ps[:].rearrange("p (h d) -> p h d", d=D))

        if variant == "qkv":
            for mt in range(CH // P):
                for qc in range(NQC):
                    emit_kproj(qc, mt)
                    emit_qproj(qc, mt)
            for tt in range(NKT):
                emit_vproj(tt)
            ot = persist.tile([P, QC], f32, tag="dump")
            nc.vector.tensor_copy(
                ot[:].rearrange("p (h d) -> p h d", d=D), v[:, 0, :, 0:D])
            nc.sync.dma_start(out_d[0:P, 0:QC], ot[:])
            return

        # ---- serial prefix: just enough for the first head's S stream ----
        for qc in range(NQC):
            emit_kproj(qc, 0)
        emit_qproj(0, 0)

        # ---- deferred projection drain, paced through the first qc ----
        # item idx i covers (qc=i//64, h=(i//8)%8, g=i%8); S(h) needs
        # kt/qt mt=h//2; PV(item i, g) needs v tiles 2g, 2g+1.
        drain_at = {}
        for i in range(NG):               # V pairs feed PV of items 0..7
            drain_at[i] = [lambda tt=2 * i: emit_vproj(tt),
                           lambda tt=2 * i + 1: emit_vproj(tt)]
        for mt in range(1, CH // P):      # K/Q for heads 2mt..2mt+1 at idx 16mt
            base = 8 * mt
            for qc in range(NQC):
                drain_at.setdefault(base + qc, []).append(
                    lambda qc=qc, mt=mt: emit_kproj(qc, mt))
            drain_at.setdefault(base + 4, []).append(
                lambda mt=mt: emit_qproj(0, mt))
        # Q(qc+1) during qc, one mt per head boundary
        for qc in range(NQC - 1):
            for mt in range(CH // P):
                drain_at.setdefault(qc * 64 + mt * 8 + 6, []).append(
                    lambda qc=qc, mt=mt: emit_qproj(qc + 1, mt))

        def emit_outproj_group(qc, i):
            qt_i, nt = 4 * qc + i // 2, i % 2
            po = ppo.tile([P, QC], f32, tag="po")
            for ct in range(CH // P):
                nc.tensor.matmul(
                    po[:], ctxt[:, ct, qt_i * P:(qt_i + 1) * P],
                    wo[:, ct, nt * QC:(nt + 1) * QC],
                    start=(ct == 0), stop=(ct == CH // P - 1))
            ot = pout.tile([P, QC], f32, tag="ot")
            nc.vector.tensor_copy(ot[:], po[:])
            nc.sync.dma_start(
                out_d[qt_i * P:(qt_i + 1) * P, nt * QC:(nt + 1) * QC], ot[:])

        if variant != "attn":
            # out-projection of qc spread across qc+1's head boundaries
            for qc in range(NQC - 1):
                for i in range(8):
                    drain_at.setdefault((qc + 1) * 64 + i * 8 + 5, []).append(
                        lambda qc=qc, i=i: emit_outproj_group(qc, i))

        flat = [(qc, h, g)
                for qc in range(NQC) for h in range(NH) for g in range(NG)]
        st_q = []

        def emit_S(it):
            qc, h, g = it
            qsl = slice(qc * QC, (qc + 1) * QC)
            hp, b64 = h // 2, (h % 2) * D
            stS = pst.tile([P, 2, QC], f32, tag="st")
            for j in range(2):
                ik = 2 * g + j
                ksl = slice(ik * P, (ik + 1) * P)
                nc.tensor.matmul(
                    stS[:, j, :],
                    kt[b64:b64 + D, hp, ksl],
                    qt[b64:b64 + D, hp, qsl],
                    start=True, stop=True, tile_position=(b64, 0))
            st_q.append(stS)

        emit_S(flat[0])
        emit_S(flat[1])
        psc = None
        for idx, it in enumerate(flat):
            qc, h, g = it
            qsl = slice(qc * QC, (qc + 1) * QC)
            stS = st_q.pop(0)
            eS = pe_pool.tile([P, 2, QC], bf16, tag="eS")
            nc.scalar.activation(eS[:], stS[:],
                                 mybir.ActivationFunctionType.Exp, scale=SCALE)
            for fn in drain_at.pop(idx, ()):
                fn()
            if idx + 2 < len(flat):
                emit_S(flat[idx + 2])
            if g == 0:
                psc = ppv.tile([P, QC], f32, tag="psc")  # bufs=1 pool
            for j in range(2):
                ik = 2 * g + j
                nc.tensor.matmul(
                    psc[0:D + 1, :], v[:, ik, h, :], eS[:, j, :],
                    start=(ik == 0), stop=(ik == NKT - 1))
            if g == NG - 1:
                # evacuate psum to SBUF (frees psc immediately), then
                # normalize via DRAM stride-0 denominator broadcast
                sctx = pnorm.tile([P, QC], f32, tag="sctx")
                nc.vector.tensor_copy(sctx[0:D + 1, :], psc[0:D + 1, :])
                den_dr = pdram.tile([QC], f32, tag="den_dr")
                nc.sync.dma_start(den_dr[:], sctx[D:D + 1, :])
                bcr = pnorm.tile([P, QC], f32, tag="bcr")
                nc.sync.dma_start(bcr[0:D, :], den_dr.partition_broadcast(D))
                bc = pnorm.tile([P, QC], f32, tag="bc")
                nc.vector.reciprocal_approx_fast(bc[0:D, :], bcr[0:D, :])
                nc.vector.tensor_mul(
                    ctxt[(h % 2) * D:(h % 2 + 1) * D, h // 2, qsl],
                    sctx[0:D, :], bc[0:D, :])

        if variant != "attn":
            for i in range(8):
                emit_outproj_group(NQC - 1, i)

        if variant == "attn":
            ot = persist.tile([P, QC], f32, tag="dump")
            nc.vector.tensor_copy(ot[:], ctxt[:, 0, 0:QC])
            nc.sync.dma_start(out_d[0:P, 0:QC], ot[:])


def _prepare_in_maps(x, wq, wk, wv, wo):
    x = np.asarray(x, dtype=np.float32)
    ws = {}
    for hg in range(HG):
        sl = slice(hg * CH, (hg + 1) * CH)
        ws[hg] = {
            "wq": np.ascontiguousarray(np.asarray(wq)[sl, :].T).astype(
                np_bf16).reshape(KO, P, CH),
            "wk": np.ascontiguousarray(np.asarray(wk)[sl, :].T).astype(
                np_bf16).reshape(KO, P, CH),
            "wv": np.ascontiguousarray(np.asarray(wv)[sl, :].T).astype(
                np_bf16).reshape(KO, P, CH),
            "wo": np.ascontiguousarray(np.asarray(wo)[:, sl].T).astype(
                np_bf16).reshape(CH // P, P, C),
        }
    in_maps = []
    for core in range(8):
        b, hg = core // HG, core % HG
        xtb = np.ascontiguousarray(x[b].T).astype(np_bf16).reshape(KO, P, N)
        m = {"xt": xtb}
        m.update(ws[hg])
        in_maps.append(m)
    return in_maps


def kernel(x, wq, wk, wv, wo, bo):
    nc = _build()
    in_maps = _prepare_in_maps(x, wq, wk, wv, wo)
    res = run_bass_kernel_spmd(nc, in_maps, core_ids=list(range(8)))
    bo = np.asarray(bo, dtype=np.float32)
    out = np.empty((B, N, C), dtype=np.float32)
    for b in range(B):
        out[b] = res.results[2 * b]["out"] + res.results[2 * b + 1]["out"] + bo
    return out
